# revision 33
# baseline (speedup 1.0000x reference)
"""Causal self-attention Trainium2 Bass kernel.

Problem: nn_CausalSelfAttention (B=2, L=2048, D=1024, H=16 heads, Khd=64).

Sharding (8 cores): data-parallel over B (2 way) x tensor-parallel over
heads (4 way, 4 heads/core).  Each core computes
  qkv_local = x_b @ W_attn_local.T          (c_attn column-sharded)
  attn_local = causal_attention(q,k,v)      (4 heads)
  y_partial  = attn_local @ W_proj_local.T  (c_proj row-sharded)
and the host sums the 4 partials per batch (the row-parallel unshard).

Device layout choices:
  - all tensors stored fp16 (matmul rate = fp32r, half DMA/SBUF, 2x DVE,
    no small-N fp32r matmul penalty); PSUM accumulation stays fp32.
  - qT,kT computed feature-major [64*4, L] directly (weights as lhsT),
    so attention score matmuls need no transposes.
  - QKV is computed per 512-column block (n-block) so attention on block
    0 starts ~4x earlier; remaining QKV n-blocks + projection pieces are
    interleaved into the attention rounds as PE filler.
  - scores computed transposed: scT[kcol, qrow] = kT.T-block @ qT-block.
  - softmax denominator via an extra ones-row appended to V (so the
    PV^T matmul also produces the row sums); 1/denominator is spread
    across partitions with a GpSimd partition_broadcast, multiply on DVE.
  - input DMA split across SP/Activation (HWDGE) and GpSimd (SWDGE)
    queues in first-needed order.
"""

import math

import numpy as np

B, L, D, H = 2, 2048, 1024, 16
KHD = D // H  # 64 head dim
NCORES = 8
HPC = 4  # heads per core
FQK = 2 * HPC * KHD  # 512 q+k local features
FV = HPC * KHD  # 256 v local features
FQKV = FQK + FV  # 768
DK = D // 128  # 8 contraction chunks
LC = L // 128  # 16 row chunks
NJ = L // 512  # 4 qrow blocks
VW = 68  # padded v_aug row width (64 feats + 1 ones + pad)
SCALE = 1.0 / math.sqrt(KHD)

_CACHE = {}


def _build(has_bqk: bool, has_bv: bool, has_bp: bool, reps: int = 1):
    import concourse.bass as bass
    import concourse.mybir as mybir
    import concourse.tile as tile
    from concourse import bacc

    f32 = mybir.dt.float32
    f16 = mybir.dt.float16

    nc = bacc.Bacc(None, target_bir_lowering=False)
    xT_d = nc.declare_dram_parameter("xT", [D, L], f16, isOutput=False)
    wqkvT_d = nc.declare_dram_parameter("wqkvT", [D, FQKV], f16, isOutput=False)
    wpT_d = nc.declare_dram_parameter("wpT", [FV, D], f16, isOutput=False)
    tri_d = nc.declare_dram_parameter("tri", [128, 128], f16, isOutput=False)
    if has_bqk:
        bqk_d = nc.declare_dram_parameter("bqk", [128, FQK // 128], f32, isOutput=False)
    if has_bv:
        bv_d = nc.declare_dram_parameter("bv", [1, FV], f16, isOutput=False)
    if has_bp:
        bp_d = nc.declare_dram_parameter("bp", [1, D], f16, isOutput=False)
    y_d = nc.declare_dram_parameter("y", [L, D], f16, isOutput=True)

    with nc.allow_low_precision(reason="fp16 storage, fp32 accumulate"), tile.TileContext(nc) as tc:
        with (
            tc.tile_pool(name="persist", bufs=1) as persist,
            tc.tile_pool(name="work", bufs=3) as work,
            tc.tile_pool(name="small", bufs=2) as small,
            tc.tile_pool(name="ps_sc", bufs=2, space="PSUM") as ps_sc,
            tc.tile_pool(name="ps_po", bufs=2, space="PSUM") as ps_po,
            tc.tile_pool(name="ps_ot", bufs=2, space="PSUM") as ps_ot,
        ):
            for _rep in range(reps):
                # ---- persistent SBUF tensors ----
                xT_sbs = [persist.tile([128, L], f16, name=f"xT{k}", tag=f"xT{k}") for k in range(DK)]
                wqkvT_sbs = [
                    persist.tile([128, FQKV], f16, name=f"wqkvT{k}", tag=f"wqkvT{k}") for k in range(DK)
                ]
                # per-plane q/k (plane p holds heads 2p, 2p+1)
                qT_p = [persist.tile([128, L], f16, name=f"qT{p}", tag=f"qT{p}") for p in range(2)]
                kT_p = [persist.tile([128, L], f16, name=f"kT{p}", tag=f"kT{p}") for p in range(2)]
                v_augs = [
                    persist.tile([128, HPC, VW], f16, name=f"vaug{lc}", tag=f"vaug{lc}")
                    for lc in range(LC)
                ]
                attnT_js = [
                    persist.tile([128, 2, 512], f16, name=f"attnT{j}", tag=f"attnT{j}")
                    for j in range(NJ)
                ]
                wpT_sbs = [persist.tile([128, D], f16, name=f"wpT{k}", tag=f"wpT{k}") for k in range(2)]
                tri_sb = persist.tile([128, 128], f16)
                if has_bv:
                    ones_row = persist.tile([1, 128], f16)
                    nc.vector.memset(ones_row, 1.0)

                # ---- input DMA, spread across queues in first-needed order ----
                # n-block 0 (q/k weights + xT cols 0:512) alternates between
                # the SP and Activation HWDGE queues so the first QKV chains
                # are fed at 2x single-queue rate.
                for k in range(DK):
                    eng = nc.sync if k % 2 == 0 else nc.scalar
                    eng.dma_start(
                        out=wqkvT_sbs[k][:, 0:FQK], in_=wqkvT_d[k * 128:(k + 1) * 128, 0:FQK]
                    )
                    eng.dma_start(
                        out=xT_sbs[k][:, 0:512], in_=xT_d[k * 128:(k + 1) * 128, 0:512]
                    )
                nc.sync.dma_start(out=tri_sb, in_=tri_d[:])
                # GpSimd (SWDGE) queue: v weights first (gate the first
                # v_chains at ~3us), then xT n-blocks 2,3 (needed latest).
                for k in range(DK):
                    nc.gpsimd.dma_start(
                        out=wqkvT_sbs[k][:, FQK:FQKV],
                        in_=wqkvT_d[k * 128:(k + 1) * 128, FQK:FQKV],
                    )
                # xT n-block 1 on SP (needed ~12us in); wp on Act.
                for k in range(DK):
                    nc.sync.dma_start(
                        out=xT_sbs[k][:, 512:1024], in_=xT_d[k * 128:(k + 1) * 128, 512:1024]
                    )
                for kc in range(2):
                    nc.scalar.dma_start(
                        out=wpT_sbs[kc], in_=wpT_d[kc * 128:(kc + 1) * 128, :]
                    )
                for k in range(DK):
                    nc.gpsimd.dma_start(
                        out=xT_sbs[k][:, 1024:2048], in_=xT_d[k * 128:(k + 1) * 128, 1024:2048]
                    )
                # ones column of v_aug (softmax denominator trick)
                for lc in range(LC):
                    nc.gpsimd.memset(v_augs[lc][:, :, KHD:KHD + 1], 1.0)
                if has_bqk:
                    bqk_sb = persist.tile([128, FQK // 128], f32)
                    nc.sync.dma_start(out=bqk_sb, in_=bqk_d[:])
                if has_bv:
                    bv_sb = persist.tile([1, FV], f16)
                    nc.sync.dma_start(out=bv_sb, in_=bv_d[:])
                if has_bp:
                    bp_sb = persist.tile([1, D], f16)
                    nc.sync.dma_start(out=bp_sb, in_=bp_d[:])

                # ---- emission helpers ----
                def qk_chain(n, m):
                    # q/k features chunk m (feats 128m..128m+127), qcol block n
                    dst = qT_p if m < 2 else kT_p
                    plane = m % 2
                    ps = ps_po.tile([128, 512], f32, tag="po", name="ps")
                    for k in range(DK):
                        nc.tensor.matmul(
                            ps,
                            wqkvT_sbs[k][:, m * 128:(m + 1) * 128],
                            xT_sbs[k][:, n * 512:(n + 1) * 512],
                            start=(k == 0),
                            stop=(k == DK - 1),
                        )
                    if has_bqk:
                        nc.scalar.activation(
                            dst[plane][:, n * 512:(n + 1) * 512],
                            ps,
                            mybir.ActivationFunctionType.Copy,
                            bias=bqk_sb[:, m:m + 1],
                        )
                    else:
                        nc.vector.tensor_copy(
                            out=dst[plane][:, n * 512:(n + 1) * 512], in_=ps
                        )

                def v_chain(lc):
                    # v natural layout [L, feat]
                    psv = ps_po.tile([128, 512], f32, tag="po", name="psv")
                    for k in range(DK):
                        nc.tensor.matmul(
                            psv[:, 0:FV],
                            xT_sbs[k][:, lc * 128:(lc + 1) * 128],
                            wqkvT_sbs[k][:, FQK:FQKV],
                            start=(k == 0),
                            stop=(k == DK - 1) and not has_bv,
                        )
                    if has_bv:
                        nc.tensor.matmul(
                            psv[:, 0:FV], ones_row[0:1, :], bv_sb,
                            start=False, stop=True,
                        )
                    nc.vector.tensor_copy(
                        out=v_augs[lc][:, :, 0:KHD],
                        in_=psv[:, 0:FV].rearrange("p (h k) -> p h k", h=HPC),
                    )

                def proj_piece(j, lq, half, alt_copy=False, alt_psum=False):
                    # projection of row chunk 4j+lq, output cols half*512..
                    lc = 4 * j + lq
                    sl = slice(half * 512, (half + 1) * 512)
                    if alt_psum:
                        # tail only: score-round PSUM slots are idle after the
                        # last exp; borrowing them deepens the piece pipeline
                        psy = ps_sc.tile([128, 1024], f32, tag="sc", name="sc")[:, 0:512]
                    else:
                        psy = ps_po.tile([128, 512], f32, tag="po", name="psy")
                    for kc in range(2):
                        nc.tensor.matmul(
                            psy,
                            attnT_js[j][:, kc, lq * 128:(lq + 1) * 128],
                            wpT_sbs[kc][:, sl],
                            start=(kc == 0),
                            stop=(kc == 1) and not has_bp,
                        )
                    if has_bp:
                        nc.tensor.matmul(
                            psy, ones_row[0:1, :], bp_sb[0:1, sl],
                            start=False, stop=True,
                        )
                    ysb = work.tile([128, 512], f16, tag="ysb", name="ysb", bufs=4)
                    if alt_copy:
                        nc.scalar.copy(ysb, psy)
                    else:
                        nc.vector.tensor_copy(out=ysb, in_=psy)
                    nc.sync.dma_start(out=y_d[lc * 128:(lc + 1) * 128, sl], in_=ysb)

                def att_pair(j, h0, fillers=()):
                    # two heads (same q/k plane) processed in lockstep, with
                    # the PV matmuls software-pipelined one round behind the
                    # scores so PE never waits on ScalarE's exp.  `fillers`
                    # are emission thunks (QKV chains / proj pieces) spliced
                    # between rounds to keep PE busy while exp runs.
                    pl = h0 // 2
                    pos = [(h0 % 2) * 64, ((h0 + 1) % 2) * 64]
                    heads = [h0, h0 + 1]
                    outTs = [
                        ps_ot.tile([128, 512], f32, tag="outT", name="outT")
                        for _ in range(2)
                    ]
                    qrs = slice(j * 512, (j + 1) * 512)
                    last_c = 4 * j + 3
                    rounds = [("below", cp) for cp in range(0, 4 * j, 2)]
                    rounds += [("diag", 0), ("diag", 2)]
                    fillers = list(fillers)
                    nfill = len(fillers)
                    pending = []

                    def flush_pending():
                        for hh, parts, ex in pending:
                            for c, exsl, n0 in parts:
                                nc.tensor.matmul(
                                    outTs[hh][0:KHD + 1, n0:512],
                                    v_augs[c][:, heads[hh], 0:KHD + 1],
                                    ex[:, exsl],
                                    start=(c == 0),
                                    stop=(c == last_c),
                                )
                        pending.clear()

                    for ri, (kind, arg) in enumerate(rounds):
                        new_pending = []
                        for hh in range(2):
                            po = pos[hh]
                            sc = ps_sc.tile([128, 1024], f32, tag="sc", name="sc")
                            if kind == "below":
                                cp = arg
                                for half in range(2):
                                    c = cp + half
                                    nc.tensor.matmul(
                                        sc[:, half * 512:(half + 1) * 512],
                                        kT_p[pl][po:po + 64, c * 128:(c + 1) * 128],
                                        qT_p[pl][po:po + 64, qrs],
                                        start=True,
                                        stop=True,
                                    )
                                ex = work.tile([128, 1024], f16, tag="expT", name="ex", bufs=5)
                                nc.scalar.activation(
                                    ex, sc,
                                    mybir.ActivationFunctionType.Exp, scale=SCALE,
                                )
                                parts = [
                                    (cp, slice(0, 512), 0),
                                    (cp + 1, slice(512, 1024), 0),
                                ]
                            else:
                                i0 = arg
                                ws = [512 - 128 * (i0 + di) for di in range(2)]
                                offs = [0, ws[0]]
                                wtot = ws[0] + ws[1]
                                for di in range(2):
                                    c = 4 * j + i0 + di
                                    n0 = 128 * (i0 + di)
                                    nc.tensor.matmul(
                                        sc[:, offs[di]:offs[di] + ws[di]],
                                        kT_p[pl][po:po + 64, c * 128:(c + 1) * 128],
                                        qT_p[pl][po:po + 64, j * 512 + n0:(j + 1) * 512],
                                        start=True,
                                        stop=True,
                                    )
                                ex = work.tile([128, 1024], f16, tag="expT", name="ex", bufs=5)
                                nc.scalar.activation(
                                    ex[:, 0:wtot], sc[:, 0:wtot],
                                    mybir.ActivationFunctionType.Exp, scale=SCALE,
                                )
                                for di in range(2):
                                    nc.vector.tensor_mul(
                                        ex[:, offs[di]:offs[di] + 128],
                                        ex[:, offs[di]:offs[di] + 128],
                                        tri_sb,
                                    )
                                parts = [
                                    (4 * j + i0, slice(0, ws[0]), 128 * i0),
                                    (
                                        4 * j + i0 + 1,
                                        slice(offs[1], offs[1] + ws[1]),
                                        128 * (i0 + 1),
                                    ),
                                ]
                            new_pending.append((hh, parts, ex))
                        # splice filler work after this round's scores (so
                        # exp can start ASAP) but BEFORE the previous
                        # round's PV flush (which waits on exp): fillers
                        # placed after the flush would sit behind the stall
                        # in PE's in-order stream.
                        f0 = nfill * ri // len(rounds)
                        f1 = nfill * (ri + 1) // len(rounds)
                        for fi in range(f0, f1):
                            fillers[fi]()
                        flush_pending()
                        pending.extend(new_pending)
                    # final flush with each head's reciprocal emitted right
                    # after that head's last PV matmul, so DVE starts head
                    # 0's recip while PE still flushes head 1
                    recips, bcs = [], []
                    for hh, parts, ex in pending:
                        for c, exsl, n0 in parts:
                            nc.tensor.matmul(
                                outTs[hh][0:KHD + 1, n0:512],
                                v_augs[c][:, heads[hh], 0:KHD + 1],
                                ex[:, exsl],
                                start=(c == 0),
                                stop=(c == last_c),
                            )
                        recip = small.tile([1, 512], f16, tag="recip", name="recip")
                        nc.vector.reciprocal(recip, outTs[hh][KHD:KHD + 1, :])
                        recips.append(recip)
                    pending.clear()
                    # normalize: attnT[f, qrow] = outT[f, qrow] / outT[64, qrow]
                    for hh in range(2):
                        bc_sb = small.tile([64, 512], f16, tag="bcsb", name="bc_sb")
                        nc.gpsimd.partition_broadcast(bc_sb, recips[hh])
                        bcs.append(bc_sb)
                    for hh in range(2):
                        nc.vector.tensor_mul(
                            attnT_js[j][pos[hh]:pos[hh] + 64, pl, :],
                            outTs[hh][0:KHD, :], bcs[hh]
                        )

                # ---- emission order: QKV n-block 0 first, attention starts
                # as soon as plane 0 of block 0 exists; later QKV n-blocks
                # and projection pieces fill PE time between score rounds ----
                def qk(n, m):
                    return lambda: qk_chain(n, m)

                def vf(lc):
                    return lambda: v_chain(lc)

                def pj(j, p):
                    return lambda: proj_piece(j, p // 2, p % 2)

                qk_chain(0, 0)
                qk_chain(0, 2)
                for lc in range(4):
                    v_chain(lc)
                att_pair(0, 0, [qk(0, 1), qk(0, 3)])
                att_pair(0, 2, [qk(1, 0), vf(4), qk(1, 2), vf(5)])
                att_pair(1, 0, [qk(1, 1), vf(6), qk(1, 3), vf(7)])
                att_pair(1, 2, [qk(2, 0), vf(8), qk(2, 2), vf(9), vf(10), vf(11)])
                att_pair(2, 0, [qk(2, 1), qk(2, 3)] + [pj(0, p) for p in range(4)])
                att_pair(2, 2, [qk(3, 0), qk(3, 2), vf(12), vf(13), vf(14), vf(15)]
                         + [pj(0, p) for p in range(4, 8)])
                att_pair(3, 0, [qk(3, 1), qk(3, 3)] + [pj(1, p) for p in range(6)])
                att_pair(3, 2, [pj(1, 6), pj(1, 7)] + [pj(2, p) for p in range(4)])
                # tail: proj(2) leftovers (independent of the final normalize)
                # hide the last pair's normalize latency; copies alternate
                # between DVE and the now-idle Activation engine.
                for p in range(4, 8):
                    proj_piece(2, p // 2, p % 2, alt_copy=(p % 2 == 1))
                for p in range(8):
                    proj_piece(3, p // 2, p % 2, alt_copy=(p % 2 == 1))

    nc.compile()
    return nc


def kernel(input_BLD, W_attn, b_attn, W_proj, b_proj):
    input_BLD = np.asarray(input_BLD, dtype=np.float32)
    W_attn = np.asarray(W_attn, dtype=np.float32)
    b_attn = np.asarray(b_attn, dtype=np.float32)
    W_proj = np.asarray(W_proj, dtype=np.float32)
    b_proj = np.asarray(b_proj, dtype=np.float32)

    has_bqk = bool(np.any(b_attn[: 2 * D]))
    has_bv = bool(np.any(b_attn[2 * D:]))
    has_bp = bool(np.any(b_proj))

    key = (has_bqk, has_bv, has_bp)
    if key not in _CACHE:
        _CACHE[key] = _build(*key)
    nc = _CACHE[key]

    tri = (np.arange(128)[None, :] >= np.arange(128)[:, None]).astype(np.float16)
    in_maps = []
    for c in range(NCORES):
        b, t = divmod(c, 4)
        hs = t * HPC * KHD  # feature offset of this core's heads
        w_loc = np.concatenate(
            [
                W_attn[hs:hs + FV],  # q rows
                W_attn[D + hs:D + hs + FV],  # k rows
                W_attn[2 * D + hs:2 * D + hs + FV],  # v rows
            ],
            axis=0,
        )  # [768, 1024]
        m = {
            "xT": np.ascontiguousarray(input_BLD[b].T.astype(np.float16)),
            "wqkvT": np.ascontiguousarray(w_loc.T.astype(np.float16)),
            "wpT": np.ascontiguousarray(W_proj[:, hs:hs + FV].T.astype(np.float16)),
            "tri": tri,
        }
        if has_bqk:
            bqk = np.concatenate([b_attn[hs:hs + FV], b_attn[D + hs:D + hs + FV]])
            m["bqk"] = np.ascontiguousarray(bqk.reshape(FQK // 128, 128).T)
        if has_bv:
            m["bv"] = b_attn[2 * D + hs:2 * D + hs + FV][None, :].astype(np.float16)
        if has_bp:
            m["bp"] = (b_proj / 4.0)[None, :].astype(np.float16)
        in_maps.append(m)

    from concourse.bass_utils import run_bass_kernel_spmd

    globals()["_last_in_maps"] = in_maps
    res = run_bass_kernel_spmd(nc, in_maps, list(range(NCORES)))
    globals()["_LAST_RESULTS"] = res
    out = np.empty((B, L, D), dtype=np.float32)
    for b in range(B):
        acc = res.results[4 * b]["y"].astype(np.float32)
        for t in range(1, 4):
            acc = acc + res.results[4 * b + t]["y"].astype(np.float32)
        out[b] = acc
    return out
